# revision 3
# baseline (speedup 1.0000x reference)
"""Gemma3 sliding-window attention layer on 8 Trainium2 NeuronCores.

Sharding: tensor-parallel over heads. Core c computes q-head c and kv-head c//2
(kv heads are duplicated across the 2 cores sharing them), then the o_proj
row-slice for its head. The 8 partial o_proj outputs are summed on the host
(the unshard step for row-sharded o_proj).

Device kernel layout choices:
- hidden is fed pre-transposed (hT [HID, S]) so the qkv matmuls contract over
  the partition dim without any on-device transpose.
- q/k are produced in [d, tok] layout (weights stationary); v in [tok, d]
  (hidden stationary). scoresT [keys, q] = kT.T-free @ qT keeps softmax sums
  and the PV contraction on the partition (keys) axis, where PE ones-matmuls
  do the reductions.
- softmax skips max-subtraction (scores are bounded: q/k are RMS-normed), so
  probs accumulate as plain exp and the 1/sum normalization folds into one
  broadcast matmul + multiply at PV evacuation.
- all matmuls run in float32r (full PE rate at free-dim >= 256).
"""
import os
import sys
import types
import contextlib
import ctypes

import numpy as np

for _p in ("/opt/trn_rl_repo", "/root/.axon_site/_ro/trn_rl_repo"):
    if os.path.isdir(_p) and _p not in sys.path:
        sys.path.insert(0, _p)

from contextlib import ExitStack

import concourse.bass as bass
import concourse.mybir as mybir
import concourse.tile as tile
from concourse import bacc
from concourse.bass_utils import run_bass_kernel_spmd

S = 4096
HID = 2560
NH = 8
NKV = 4
HD = 256
WIN = 1024
ROPE_BASE = 10000.0
EPS = 1e-6
SCALING = HD ** -0.5

NCORES = 8
CH = 256            # tokens per chunk/block
NCH = S // CH       # 16
KT = HID // 128     # 20 hid k-tiles
f32 = mybir.dt.float32
f32r = mybir.dt.float32r
AF = mybir.ActivationFunctionType

_NC = None
_last_results = None


def _install_ntff_shim():
    """antenv.axon_hooks is absent in this image; rebuild it over libaxon so
    run_bass_kernel_spmd(trace=True) can capture NTFF profiles."""
    if "antenv.axon_hooks" in sys.modules:
        return
    so_path = "/opt/axon/libaxon_pjrt.so"
    hook = None
    try:
        lib = ctypes.CDLL(so_path)
        if hasattr(lib, "axon_start_nrt_profile"):
            lib.axon_start_nrt_profile.argtypes = [
                ctypes.POINTER(ctypes.c_int64),
                ctypes.c_size_t,
            ]
            lib.axon_start_nrt_profile.restype = ctypes.c_int64
            lib.axon_stop_nrt_profile.argtypes = [ctypes.c_char_p]
            lib.axon_stop_nrt_profile.restype = ctypes.c_int64

            @contextlib.contextmanager
            def _hook(output_dir, device_ids):
                import jax

                jax.devices()
                if device_ids:
                    ids = (ctypes.c_int64 * len(device_ids))(*device_ids)
                    rc = lib.axon_start_nrt_profile(ids, len(device_ids))
                else:
                    rc = lib.axon_start_nrt_profile(None, 0)
                if rc != 0:
                    raise RuntimeError(f"axon_start_nrt_profile rc={rc}")
                try:
                    yield
                finally:
                    n = lib.axon_stop_nrt_profile(str(output_dir).encode())
                    if n < 0:
                        raise RuntimeError(f"axon_stop_nrt_profile rc={n}")

            hook = _hook
    except OSError:
        pass
    mod = types.ModuleType("antenv.axon_hooks")
    mod.get_axon_ntff_profile_hook = lambda: hook
    mod.set_axon_ntff_profile_hook = lambda h: None
    sys.modules["antenv.axon_hooks"] = mod


def _body(ctx, tc, hT, w, ow, cs, msk, nw, on, on1, outp):
    nc = tc.nc

    const = ctx.enter_context(tc.tile_pool(name="const", bufs=1))
    hpool = ctx.enter_context(tc.tile_pool(name="hT", bufs=2))
    cspool = ctx.enter_context(tc.tile_pool(name="cs", bufs=2))
    qpool = ctx.enter_context(tc.tile_pool(name="qT", bufs=2))
    kvpool = ctx.enter_context(tc.tile_pool(name="kv", bufs=6))
    vpool = ctx.enter_context(tc.tile_pool(name="v", bufs=12))
    tmp = ctx.enter_context(tc.tile_pool(name="tmp", bufs=3))
    sqpool = ctx.enter_context(tc.tile_pool(name="sq", bufs=4))
    small = ctx.enter_context(tc.tile_pool(name="small", bufs=3))
    ppool = ctx.enter_context(tc.tile_pool(name="probs", bufs=3))
    apool = ctx.enter_context(tc.tile_pool(name="attnT", bufs=4))
    opool = ctx.enter_context(tc.tile_pool(name="osb", bufs=3))

    rot = ctx.enter_context(tc.tile_pool(name="rot", bufs=5, space="PSUM"))
    pvp = ctx.enter_context(tc.tile_pool(name="pv", bufs=2, space="PSUM"))
    smp = ctx.enter_context(tc.tile_pool(name="sums", bufs=1, space="PSUM"))

    # resident constants
    w_sb = const.tile([128, KT * 768], f32r)
    for k in range(KT):
        nc.sync.dma_start(out=w_sb[:, k * 768:(k + 1) * 768],
                          in_=w[k * 128:(k + 1) * 128, :])
    ow_sb = const.tile([128, 2 * HID], f32r)
    for h in range(2):
        nc.sync.dma_start(out=ow_sb[:, h * HID:(h + 1) * HID],
                          in_=ow[h * 128:(h + 1) * 128, :])
    msk_sb = const.tile([128, 384], f32)
    nc.sync.dma_start(out=msk_sb, in_=msk)
    nw_sb = const.tile([128, 4], f32)
    nc.sync.dma_start(out=nw_sb, in_=nw)
    ones_sb = const.tile([128, 2], f32r)
    nc.sync.dma_start(out=ones_sb, in_=on)
    ones1_sb = const.tile([1, 128], f32r)
    nc.sync.dma_start(out=ones1_sb, in_=on1)

    kv_tiles = {}
    v_tiles = {}

    for t in range(NCH):
        t0 = t * CH

        hTt = hpool.tile([128, KT * CH], f32r, tag="hTt")
        for k in range(KT):
            nc.sync.dma_start(out=hTt[:, k * CH:(k + 1) * CH],
                              in_=hT[k * 128:(k + 1) * 128, t0:t0 + CH])
        cst = cspool.tile([128, 2 * CH], f32, tag="cst")
        nc.sync.dma_start(out=cst[:, 0:CH], in_=cs[:, t0:t0 + CH])
        nc.sync.dma_start(out=cst[:, CH:2 * CH], in_=cs[:, S + t0:S + t0 + CH])
        cos = cst[:, 0:CH]
        sin = cst[:, CH:2 * CH]

        qTt = qpool.tile([128, 2 * CH], f32r, tag="qTt")
        kvt = kvpool.tile([128, 2 * CH], f32r, tag="kvt")

        # q then k: projection -> rmsnorm -> rope, output [d, tok]
        for (j0, wo, dest) in ((0, 0, qTt), (2, 2, kvt)):
            xps = []
            for j in (j0, j0 + 1):
                ps = rot.tile([128, CH], f32, tag="rot")
                for k in range(KT):
                    nc.tensor.matmul(
                        ps,
                        w_sb[:, k * 768 + j * 128:k * 768 + (j + 1) * 128],
                        hTt[:, k * CH:(k + 1) * CH],
                        start=(k == 0), stop=(k == KT - 1))
                xps.append(ps)
            x0p, x1p = xps
            # sum of squares over head_dim via ones-matmul
            sq0 = sqpool.tile([128, CH], f32r, tag="sq")
            sq1 = sqpool.tile([128, CH], f32r, tag="sq")
            nc.scalar.activation(sq0, x0p, AF.Square)
            nc.scalar.activation(sq1, x1p, AF.Square)
            ssq = rot.tile([1, CH], f32, tag="rot")
            nc.tensor.matmul(ssq, ones_sb[:, 0:1], sq0, start=True, stop=False)
            nc.tensor.matmul(ssq, ones_sb[:, 0:1], sq1, start=False, stop=True)
            t1 = small.tile([1, CH], f32, tag="sm1")
            nc.scalar.activation(t1, ssq, AF.Copy, bias=EPS, scale=1.0 / HD)
            t2 = small.tile([1, CH], f32, tag="sm2")
            nc.vector.reciprocal(t2, t1)
            rstd = small.tile([1, CH], f32r, tag="sm3")
            nc.scalar.activation(rstd, t2, AF.Sqrt)
            rb = rot.tile([128, CH], f32, tag="rot")
            nc.tensor.matmul(rb, ones1_sb, rstd, start=True, stop=True)
            # evacuate x*(1+w) from psum on ACT
            x0 = tmp.tile([128, CH], f32, tag="x")
            x1 = tmp.tile([128, CH], f32, tag="x")
            nc.scalar.activation(x0, x0p, AF.Copy, bias=0.0,
                                 scale=nw_sb[:, wo:wo + 1])
            nc.scalar.activation(x1, x1p, AF.Copy, bias=0.0,
                                 scale=nw_sb[:, wo + 1:wo + 2])
            # rope mix
            a = tmp.tile([128, CH], f32, tag="m")
            nc.vector.tensor_mul(a, x0, cos)
            b = tmp.tile([128, CH], f32, tag="m")
            nc.vector.tensor_mul(b, x1, sin)
            e = tmp.tile([128, CH], f32, tag="m")
            nc.vector.tensor_sub(e, a, b)
            c_ = tmp.tile([128, CH], f32, tag="m")
            nc.vector.tensor_mul(c_, x1, cos)
            d = tmp.tile([128, CH], f32, tag="m")
            nc.vector.tensor_mul(d, x0, sin)
            f = tmp.tile([128, CH], f32, tag="m")
            nc.vector.tensor_add(f, c_, d)
            nc.vector.tensor_mul(dest[:, 0:CH], e, rb)
            nc.vector.tensor_mul(dest[:, CH:2 * CH], f, rb)
        kv_tiles[t] = kvt

        # v projection, natural [tok, d] layout
        for st in range(2):
            vps = rot.tile([128, HD], f32, tag="rot")
            for k in range(KT):
                nc.tensor.matmul(
                    vps,
                    hTt[:, k * CH + st * 128:k * CH + st * 128 + 128],
                    w_sb[:, k * 768 + 512:(k + 1) * 768],
                    start=(k == 0), stop=(k == KT - 1))
            vt = vpool.tile([128, HD], f32r, tag="v")
            nc.scalar.activation(vt, vps, AF.Copy, bias=0.0, scale=1.0)
            v_tiles[2 * t + st] = vt

        # attention for the 256 queries of this block
        pv0 = pvp.tile([128, CH], f32, tag="pv")
        pv1 = pvp.tile([128, CH], f32, tag="pv")
        sums = smp.tile([1, CH], f32, tag="sums")
        kts = list(range(max(0, 2 * t - 8), 2 * t + 2))
        for i, kt in enumerate(kts):
            ct, sb = kt // 2, kt % 2
            kvsrc = kv_tiles[ct]
            sc = rot.tile([128, CH], f32, tag="rot")
            for h in range(2):
                nc.tensor.matmul(
                    sc,
                    kvsrc[:, h * CH + sb * 128:h * CH + sb * 128 + 128],
                    qTt[:, h * CH:(h + 1) * CH],
                    start=(h == 0), stop=(h == 1))
            pr = ppool.tile([128, CH], f32r, tag="pr")
            nc.scalar.activation(pr, sc, AF.Exp, bias=0.0, scale=SCALING)
            for sidx, qt in enumerate((2 * t, 2 * t + 1)):
                sl = slice(sidx * 128, (sidx + 1) * 128)
                if kt == qt:
                    m = msk_sb[:, 256:384]
                elif kt > qt or kt < qt - 8:
                    m = msk_sb[:, 128:256]
                elif kt == qt - 8:
                    m = msk_sb[:, 0:128]
                else:
                    m = None
                if m is not None:
                    nc.vector.tensor_mul(pr[:, sl], pr[:, sl], m)
            first, last = (i == 0), (i == len(kts) - 1)
            nc.tensor.matmul(sums, ones_sb[:, 0:1], pr,
                             start=first, stop=last)
            vt = v_tiles[kt]
            nc.tensor.matmul(pv0, vt[:, 0:128], pr, start=first, stop=last)
            nc.tensor.matmul(pv1, vt[:, 128:256], pr, start=first, stop=last)

        inv = small.tile([1, CH], f32r, tag="sm4")
        nc.vector.reciprocal(inv, sums)
        ib = rot.tile([128, CH], f32, tag="rot")
        nc.tensor.matmul(ib, ones1_sb, inv, start=True, stop=True)
        ibs = tmp.tile([128, CH], f32, tag="ibs")
        nc.scalar.activation(ibs, ib, AF.Copy, bias=0.0, scale=1.0)
        at0 = apool.tile([128, CH], f32r, tag="at")
        at1 = apool.tile([128, CH], f32r, tag="at")
        nc.vector.tensor_mul(at0, pv0, ibs)
        nc.vector.tensor_mul(at1, pv1, ibs)

        # o_proj row-slice: partial [256 tok, HID]
        for st in range(2):
            for hc in range(HID // 512):
                op = rot.tile([128, 512], f32, tag="rot")
                nc.tensor.matmul(op, at0[:, st * 128:(st + 1) * 128],
                                 ow_sb[:, hc * 512:(hc + 1) * 512],
                                 start=True, stop=False)
                nc.tensor.matmul(op, at1[:, st * 128:(st + 1) * 128],
                                 ow_sb[:, HID + hc * 512:HID + (hc + 1) * 512],
                                 start=False, stop=True)
                ob = opool.tile([128, 512], f32, tag="ob")
                nc.vector.tensor_copy(ob, op)
                nc.sync.dma_start(
                    out=outp[t0 + st * 128:t0 + (st + 1) * 128,
                             hc * 512:(hc + 1) * 512],
                    in_=ob)


def _build():
    nc = bacc.Bacc("TRN2", target_bir_lowering=False, debug=False,
                   num_devices=NCORES)
    hT = nc.dram_tensor("hT", [HID, S], f32r, kind="ExternalInput").ap()
    w = nc.dram_tensor("w", [HID, 768], f32r, kind="ExternalInput").ap()
    ow = nc.dram_tensor("ow", [HD, HID], f32r, kind="ExternalInput").ap()
    cs = nc.dram_tensor("cs", [128, 2 * S], f32, kind="ExternalInput").ap()
    msk = nc.dram_tensor("msk", [128, 384], f32, kind="ExternalInput").ap()
    nw = nc.dram_tensor("nw", [128, 4], f32, kind="ExternalInput").ap()
    on = nc.dram_tensor("on", [128, 2], f32r, kind="ExternalInput").ap()
    on1 = nc.dram_tensor("on1", [1, 128], f32r, kind="ExternalInput").ap()
    outp = nc.dram_tensor("outp", [S, HID], f32, kind="ExternalOutput").ap()
    with tile.TileContext(nc) as tc, ExitStack() as ctx:
        with nc.allow_low_precision(reason="float32r matmul pipeline"):
            _body(ctx, tc, hT, w, ow, cs, msk, nw, on, on1, outp)
    nc.compile()
    return nc


def _get_nc():
    global _NC
    if _NC is None:
        _NC = _build()
    return _NC


def kernel(positions, hidden_states, qkv_w, o_w, q_norm_w, k_norm_w):
    global _last_results
    _install_ntff_shim()

    positions = np.asarray(positions)
    hidden_states = np.asarray(hidden_states, dtype=np.float32)
    qkv_w = np.asarray(qkv_w, dtype=np.float32)
    o_w = np.asarray(o_w, dtype=np.float32)
    q_norm_w = np.asarray(q_norm_w, dtype=np.float32)
    k_norm_w = np.asarray(k_norm_w, dtype=np.float32)
    assert np.array_equal(positions.astype(np.int64), np.arange(S)), \
        "kernel assumes contiguous arange positions (banded sliding window)"

    hT = np.ascontiguousarray(hidden_states.T)  # [HID, S]

    inv_freq = 1.0 / (ROPE_BASE ** (np.arange(0, HD, 2, dtype=np.float32) / HD))
    freqs = positions.astype(np.float32)[:, None] * inv_freq[None, :]  # [S,128]
    cos_t = np.ascontiguousarray(np.cos(freqs).T.astype(np.float32))
    sin_t = np.ascontiguousarray(np.sin(freqs).T.astype(np.float32))
    cs = np.concatenate([cos_t, sin_t], axis=1)  # [128, 2S]

    kl = np.arange(128)[:, None]
    ql = np.arange(128)[None, :]
    edge = (kl > ql).astype(np.float32)
    diag = (kl <= ql).astype(np.float32)
    zero = np.zeros((128, 128), np.float32)
    msk = np.concatenate([edge, zero, diag], axis=1)  # [128, 384]

    nwq = 1.0 + q_norm_w
    nwk = 1.0 + k_norm_w
    nw = np.stack([nwq[:128], nwq[128:], nwk[:128], nwk[128:]], axis=1)
    nw = np.ascontiguousarray(nw.astype(np.float32))  # [128, 4]

    on = np.ones((128, 2), np.float32)
    on1 = np.ones((1, 128), np.float32)

    in_maps = []
    for c in range(NCORES):
        g = c // 2
        wq = qkv_w[:, c * HD:(c + 1) * HD]
        wk = qkv_w[:, NH * HD + g * HD:NH * HD + (g + 1) * HD]
        wv = qkv_w[:, (NH + NKV) * HD + g * HD:(NH + NKV) * HD + (g + 1) * HD]
        wslice = np.ascontiguousarray(
            np.concatenate([wq, wk, wv], axis=1).astype(np.float32))
        owslice = np.ascontiguousarray(
            o_w[c * HD:(c + 1) * HD, :].astype(np.float32))
        in_maps.append({
            "hT": hT, "w": wslice, "ow": owslice, "cs": cs, "msk": msk,
            "nw": nw, "on": on, "on1": on1,
        })

    nc = _get_nc()
    res = run_bass_kernel_spmd(nc, in_maps, list(range(NCORES)))
    _last_results = res

    out = res.results[0]["outp"].astype(np.float32).copy()
    for c in range(1, NCORES):
        out += res.results[c]["outp"]
    return out


# revision 4
# speedup vs baseline: 1.0135x; 1.0135x over previous
"""Gemma3 sliding-window attention layer on 8 Trainium2 NeuronCores.

Sharding: tensor-parallel over heads. Core c computes q-head c and kv-head c//2
(kv heads are duplicated across the 2 cores sharing them), then the o_proj
row-slice for its head. The 8 partial o_proj outputs are summed on the host
(the unshard step for row-sharded o_proj).

Device kernel layout choices:
- hidden is fed pre-transposed (hT [HID, S]) so the qkv matmuls contract over
  the partition dim without any on-device transpose.
- q/k are produced in [d, tok] layout (weights stationary); v in [tok, d]
  (hidden stationary). scoresT [keys, q] = kT.T-free @ qT keeps softmax sums
  and the PV contraction on the partition (keys) axis, where PE ones-matmuls
  do the reductions.
- softmax skips max-subtraction (scores are bounded: q/k are RMS-normed), so
  probs accumulate as plain exp and the 1/sum normalization folds into one
  broadcast matmul + multiply at PV evacuation.
- all matmuls run in float32r (full PE rate at free-dim >= 256).
"""
import os
import sys
import types
import contextlib
import ctypes

import numpy as np

for _p in ("/opt/trn_rl_repo", "/root/.axon_site/_ro/trn_rl_repo"):
    if os.path.isdir(_p) and _p not in sys.path:
        sys.path.insert(0, _p)

from contextlib import ExitStack

import concourse.bass as bass
import concourse.mybir as mybir
import concourse.tile as tile
from concourse import bacc
from concourse.bass_utils import run_bass_kernel_spmd

S = 4096
HID = 2560
NH = 8
NKV = 4
HD = 256
WIN = 1024
ROPE_BASE = 10000.0
EPS = 1e-6
SCALING = HD ** -0.5

NCORES = 8
CH = 256            # tokens per chunk/block
NCH = S // CH       # 16
KT = HID // 128     # 20 hid k-tiles
f32 = mybir.dt.float32
f32r = mybir.dt.float32r
AF = mybir.ActivationFunctionType

_NC = None
_last_results = None


def _install_ntff_shim():
    """antenv.axon_hooks is absent in this image; rebuild it over libaxon so
    run_bass_kernel_spmd(trace=True) can capture NTFF profiles."""
    if "antenv.axon_hooks" in sys.modules:
        return
    so_path = "/opt/axon/libaxon_pjrt.so"
    hook = None
    try:
        lib = ctypes.CDLL(so_path)
        if hasattr(lib, "axon_start_nrt_profile"):
            lib.axon_start_nrt_profile.argtypes = [
                ctypes.POINTER(ctypes.c_int64),
                ctypes.c_size_t,
            ]
            lib.axon_start_nrt_profile.restype = ctypes.c_int64
            lib.axon_stop_nrt_profile.argtypes = [ctypes.c_char_p]
            lib.axon_stop_nrt_profile.restype = ctypes.c_int64

            @contextlib.contextmanager
            def _hook(output_dir, device_ids):
                import jax

                jax.devices()
                if device_ids:
                    ids = (ctypes.c_int64 * len(device_ids))(*device_ids)
                    rc = lib.axon_start_nrt_profile(ids, len(device_ids))
                else:
                    rc = lib.axon_start_nrt_profile(None, 0)
                if rc != 0:
                    raise RuntimeError(f"axon_start_nrt_profile rc={rc}")
                try:
                    yield
                finally:
                    n = lib.axon_stop_nrt_profile(str(output_dir).encode())
                    if n < 0:
                        raise RuntimeError(f"axon_stop_nrt_profile rc={n}")

            hook = _hook
    except OSError:
        pass
    mod = types.ModuleType("antenv.axon_hooks")
    mod.get_axon_ntff_profile_hook = lambda: hook
    mod.set_axon_ntff_profile_hook = lambda h: None
    sys.modules["antenv.axon_hooks"] = mod


def _body(ctx, tc, hT, w, ow, cs, msk, nw, on, on1, outp):
    nc = tc.nc

    const = ctx.enter_context(tc.tile_pool(name="const", bufs=1))
    hpool = ctx.enter_context(tc.tile_pool(name="hT", bufs=2))
    cspool = ctx.enter_context(tc.tile_pool(name="cs", bufs=2))
    qpool = ctx.enter_context(tc.tile_pool(name="qT", bufs=2))
    kvpool = ctx.enter_context(tc.tile_pool(name="kv", bufs=6))
    vpool = ctx.enter_context(tc.tile_pool(name="v", bufs=12))
    tmp = ctx.enter_context(tc.tile_pool(name="tmp", bufs=3))
    sqpool = ctx.enter_context(tc.tile_pool(name="sq", bufs=2))
    small = ctx.enter_context(tc.tile_pool(name="small", bufs=3))
    ppool = ctx.enter_context(tc.tile_pool(name="probs", bufs=2))
    apool = ctx.enter_context(tc.tile_pool(name="attnT", bufs=3))
    opool = ctx.enter_context(tc.tile_pool(name="osb", bufs=2))

    rot = ctx.enter_context(tc.tile_pool(name="rot", bufs=5, space="PSUM"))
    pvp = ctx.enter_context(tc.tile_pool(name="pv", bufs=2, space="PSUM"))
    smp = ctx.enter_context(tc.tile_pool(name="sums", bufs=1, space="PSUM"))

    # resident constants
    w_sb = const.tile([128, KT * 768], f32r)
    nc.sync.dma_start(out=w_sb, in_=w)
    ow_sb = const.tile([128, 2 * HID], f32r)
    nc.sync.dma_start(out=ow_sb, in_=ow)
    msk_sb = const.tile([128, 384], f32)
    nc.sync.dma_start(out=msk_sb, in_=msk)
    nw_sb = const.tile([128, 4], f32)
    nc.sync.dma_start(out=nw_sb, in_=nw)
    ones_sb = const.tile([128, 2], f32r)
    nc.sync.dma_start(out=ones_sb, in_=on)
    ones1_sb = const.tile([1, 128], f32r)
    nc.sync.dma_start(out=ones1_sb, in_=on1)

    kv_tiles = {}
    v_tiles = {}

    for t in range(NCH):
        t0 = t * CH

        hTt = hpool.tile([128, KT * CH], f32r, tag="hTt")
        nc.sync.dma_start(out=hTt, in_=hT[:, t * KT * CH:(t + 1) * KT * CH])
        cst = cspool.tile([128, 2 * CH], f32, tag="cst")
        nc.sync.dma_start(out=cst, in_=cs[:, t * 2 * CH:(t + 1) * 2 * CH])
        cos = cst[:, 0:CH]
        sin = cst[:, CH:2 * CH]

        qTt = qpool.tile([128, 2 * CH], f32r, tag="qTt")
        kvt = kvpool.tile([128, 2 * CH], f32r, tag="kvt")

        # q then k: projection -> rmsnorm -> rope, output [d, tok]
        for (j0, wo, dest) in ((0, 0, qTt), (2, 2, kvt)):
            xps = []
            for j in (j0, j0 + 1):
                ps = rot.tile([128, CH], f32, tag="rot")
                for k in range(KT):
                    nc.tensor.matmul(
                        ps,
                        w_sb[:, k * 768 + j * 128:k * 768 + (j + 1) * 128],
                        hTt[:, k * CH:(k + 1) * CH],
                        start=(k == 0), stop=(k == KT - 1))
                xps.append(ps)
            x0p, x1p = xps
            # sum of squares over head_dim via ones-matmul
            sq0 = sqpool.tile([128, CH], f32r, tag="sq")
            sq1 = sqpool.tile([128, CH], f32r, tag="sq")
            nc.scalar.activation(sq0, x0p, AF.Square)
            nc.scalar.activation(sq1, x1p, AF.Square)
            ssq = rot.tile([1, CH], f32, tag="rot")
            nc.tensor.matmul(ssq, ones_sb[:, 0:1], sq0, start=True, stop=False)
            nc.tensor.matmul(ssq, ones_sb[:, 0:1], sq1, start=False, stop=True)
            t1 = small.tile([1, CH], f32, tag="sm1")
            nc.scalar.activation(t1, ssq, AF.Copy, bias=EPS, scale=1.0 / HD)
            t2 = small.tile([1, CH], f32, tag="sm2")
            nc.vector.reciprocal(t2, t1)
            rstd = small.tile([1, CH], f32r, tag="sm3")
            nc.scalar.activation(rstd, t2, AF.Sqrt)
            rb = rot.tile([128, CH], f32, tag="rot")
            nc.tensor.matmul(rb, ones1_sb, rstd, start=True, stop=True)
            # evacuate x*(1+w) from psum on ACT
            x0 = tmp.tile([128, CH], f32, tag="x")
            x1 = tmp.tile([128, CH], f32, tag="x")
            nc.scalar.activation(x0, x0p, AF.Copy, bias=0.0,
                                 scale=nw_sb[:, wo:wo + 1])
            nc.scalar.activation(x1, x1p, AF.Copy, bias=0.0,
                                 scale=nw_sb[:, wo + 1:wo + 2])
            # rope mix
            a = tmp.tile([128, CH], f32, tag="m")
            nc.vector.tensor_mul(a, x0, cos)
            b = tmp.tile([128, CH], f32, tag="m")
            nc.vector.tensor_mul(b, x1, sin)
            e = tmp.tile([128, CH], f32, tag="m")
            nc.vector.tensor_sub(e, a, b)
            c_ = tmp.tile([128, CH], f32, tag="m")
            nc.vector.tensor_mul(c_, x1, cos)
            d = tmp.tile([128, CH], f32, tag="m")
            nc.vector.tensor_mul(d, x0, sin)
            f = tmp.tile([128, CH], f32, tag="m")
            nc.vector.tensor_add(f, c_, d)
            nc.vector.tensor_mul(dest[:, 0:CH], e, rb)
            nc.vector.tensor_mul(dest[:, CH:2 * CH], f, rb)
        kv_tiles[t] = kvt

        # v projection, natural [tok, d] layout
        for st in range(2):
            vps = rot.tile([128, HD], f32, tag="rot")
            for k in range(KT):
                nc.tensor.matmul(
                    vps,
                    hTt[:, k * CH + st * 128:k * CH + st * 128 + 128],
                    w_sb[:, k * 768 + 512:(k + 1) * 768],
                    start=(k == 0), stop=(k == KT - 1))
            vt = vpool.tile([128, HD], f32r, tag="v")
            nc.scalar.activation(vt, vps, AF.Copy, bias=0.0, scale=1.0)
            v_tiles[2 * t + st] = vt

        # attention for the 256 queries of this block
        pv0 = pvp.tile([128, CH], f32, tag="pv")
        pv1 = pvp.tile([128, CH], f32, tag="pv")
        sums = smp.tile([1, CH], f32, tag="sums")
        kts = list(range(max(0, 2 * t - 8), 2 * t + 2))
        for i, kt in enumerate(kts):
            ct, sb = kt // 2, kt % 2
            kvsrc = kv_tiles[ct]
            sc = rot.tile([128, CH], f32, tag="rot")
            for h in range(2):
                nc.tensor.matmul(
                    sc,
                    kvsrc[:, h * CH + sb * 128:h * CH + sb * 128 + 128],
                    qTt[:, h * CH:(h + 1) * CH],
                    start=(h == 0), stop=(h == 1))
            pr = ppool.tile([128, CH], f32r, tag="pr")
            nc.scalar.activation(pr, sc, AF.Exp, bias=0.0, scale=SCALING)
            for sidx, qt in enumerate((2 * t, 2 * t + 1)):
                sl = slice(sidx * 128, (sidx + 1) * 128)
                if kt == qt:
                    m = msk_sb[:, 256:384]
                elif kt > qt or kt < qt - 8:
                    m = msk_sb[:, 128:256]
                elif kt == qt - 8:
                    m = msk_sb[:, 0:128]
                else:
                    m = None
                if m is not None:
                    nc.vector.tensor_mul(pr[:, sl], pr[:, sl], m)
            first, last = (i == 0), (i == len(kts) - 1)
            nc.tensor.matmul(sums, ones_sb[:, 0:1], pr,
                             start=first, stop=last)
            vt = v_tiles[kt]
            nc.tensor.matmul(pv0, vt[:, 0:128], pr, start=first, stop=last)
            nc.tensor.matmul(pv1, vt[:, 128:256], pr, start=first, stop=last)

        inv = small.tile([1, CH], f32r, tag="sm4")
        nc.vector.reciprocal(inv, sums)
        ib = rot.tile([128, CH], f32, tag="rot")
        nc.tensor.matmul(ib, ones1_sb, inv, start=True, stop=True)
        ibs = tmp.tile([128, CH], f32, tag="ibs")
        nc.scalar.activation(ibs, ib, AF.Copy, bias=0.0, scale=1.0)
        at0 = apool.tile([128, CH], f32r, tag="at")
        at1 = apool.tile([128, CH], f32r, tag="at")
        nc.vector.tensor_mul(at0, pv0, ibs)
        nc.vector.tensor_mul(at1, pv1, ibs)

        # o_proj row-slice: partial [256 tok, HID]
        for st in range(2):
            ob = opool.tile([128, HID], f32, tag="ob")
            for hc in range(HID // 512):
                op = rot.tile([128, 512], f32, tag="rot")
                nc.tensor.matmul(op, at0[:, st * 128:(st + 1) * 128],
                                 ow_sb[:, hc * 512:(hc + 1) * 512],
                                 start=True, stop=False)
                nc.tensor.matmul(op, at1[:, st * 128:(st + 1) * 128],
                                 ow_sb[:, HID + hc * 512:HID + (hc + 1) * 512],
                                 start=False, stop=True)
                nc.vector.tensor_copy(ob[:, hc * 512:(hc + 1) * 512], op)
            nc.sync.dma_start(
                out=outp[t0 + st * 128:t0 + (st + 1) * 128, :], in_=ob)


def _build():
    nc = bacc.Bacc("TRN2", target_bir_lowering=False, debug=False,
                   num_devices=NCORES)
    hT = nc.dram_tensor("hT", [128, KT * S], f32r, kind="ExternalInput").ap()
    w = nc.dram_tensor("w", [128, KT * 768], f32r, kind="ExternalInput").ap()
    ow = nc.dram_tensor("ow", [128, 2 * HID], f32r, kind="ExternalInput").ap()
    cs = nc.dram_tensor("cs", [128, NCH * 2 * CH], f32, kind="ExternalInput").ap()
    msk = nc.dram_tensor("msk", [128, 384], f32, kind="ExternalInput").ap()
    nw = nc.dram_tensor("nw", [128, 4], f32, kind="ExternalInput").ap()
    on = nc.dram_tensor("on", [128, 2], f32r, kind="ExternalInput").ap()
    on1 = nc.dram_tensor("on1", [1, 128], f32r, kind="ExternalInput").ap()
    outp = nc.dram_tensor("outp", [S, HID], f32, kind="ExternalOutput").ap()
    with tile.TileContext(nc) as tc, ExitStack() as ctx:
        with nc.allow_low_precision(reason="float32r matmul pipeline"):
            _body(ctx, tc, hT, w, ow, cs, msk, nw, on, on1, outp)
    nc.compile()
    return nc


def _get_nc():
    global _NC
    if _NC is None:
        _NC = _build()
    return _NC


def kernel(positions, hidden_states, qkv_w, o_w, q_norm_w, k_norm_w):
    global _last_results
    _install_ntff_shim()

    positions = np.asarray(positions)
    hidden_states = np.asarray(hidden_states, dtype=np.float32)
    qkv_w = np.asarray(qkv_w, dtype=np.float32)
    o_w = np.asarray(o_w, dtype=np.float32)
    q_norm_w = np.asarray(q_norm_w, dtype=np.float32)
    k_norm_w = np.asarray(k_norm_w, dtype=np.float32)
    assert np.array_equal(positions.astype(np.int64), np.arange(S)), \
        "kernel assumes contiguous arange positions (banded sliding window)"

    hT0 = hidden_states.T  # [HID, S]
    hT = np.ascontiguousarray(
        hT0.reshape(KT, 128, NCH, CH).transpose(1, 2, 0, 3).reshape(128, KT * S))

    inv_freq = 1.0 / (ROPE_BASE ** (np.arange(0, HD, 2, dtype=np.float32) / HD))
    freqs = positions.astype(np.float32)[:, None] * inv_freq[None, :]  # [S,128]
    cos_t = np.ascontiguousarray(np.cos(freqs).T.astype(np.float32))
    sin_t = np.ascontiguousarray(np.sin(freqs).T.astype(np.float32))
    csb = np.stack([cos_t.reshape(128, NCH, CH), sin_t.reshape(128, NCH, CH)],
                   axis=2)  # [128, NCH, 2, CH]
    cs = np.ascontiguousarray(csb.reshape(128, NCH * 2 * CH))

    kl = np.arange(128)[:, None]
    ql = np.arange(128)[None, :]
    edge = (kl > ql).astype(np.float32)
    diag = (kl <= ql).astype(np.float32)
    zero = np.zeros((128, 128), np.float32)
    msk = np.concatenate([edge, zero, diag], axis=1)  # [128, 384]

    nwq = 1.0 + q_norm_w
    nwk = 1.0 + k_norm_w
    nw = np.stack([nwq[:128], nwq[128:], nwk[:128], nwk[128:]], axis=1)
    nw = np.ascontiguousarray(nw.astype(np.float32))  # [128, 4]

    on = np.ones((128, 2), np.float32)
    on1 = np.ones((1, 128), np.float32)

    in_maps = []
    for c in range(NCORES):
        g = c // 2
        wq = qkv_w[:, c * HD:(c + 1) * HD]
        wk = qkv_w[:, NH * HD + g * HD:NH * HD + (g + 1) * HD]
        wv = qkv_w[:, (NH + NKV) * HD + g * HD:(NH + NKV) * HD + (g + 1) * HD]
        wslice = np.concatenate([wq, wk, wv], axis=1).astype(np.float32)
        wslice = np.ascontiguousarray(
            wslice.reshape(KT, 128, 768).transpose(1, 0, 2).reshape(128, KT * 768))
        owslice = o_w[c * HD:(c + 1) * HD, :].astype(np.float32)
        owslice = np.ascontiguousarray(
            owslice.reshape(2, 128, HID).transpose(1, 0, 2).reshape(128, 2 * HID))
        in_maps.append({
            "hT": hT, "w": wslice, "ow": owslice, "cs": cs, "msk": msk,
            "nw": nw, "on": on, "on1": on1,
        })

    nc = _get_nc()
    res = run_bass_kernel_spmd(nc, in_maps, list(range(NCORES)))
    _last_results = res

    out = res.results[0]["outp"].astype(np.float32).copy()
    for c in range(1, NCORES):
        out += res.results[c]["outp"]
    return out
